# revision 4
# baseline (speedup 1.0000x reference)
"""Trainium2 Bass kernel for the Brill-Lindquist Christoffel-symbol grid.

Math: the reference reduces to
    psi  = 1 + sum_n m_n / (2 r_n),   m = softplus(pre)
    h    = psi^4
    G_c  = finite-difference gradient of h along grid axis c (2nd order
           central interior, 1st order one-sided edges, spacing DX)
    W_c  = 0.5 * G_c / h
    Gamma^i_{jk} = delta_ij W_k + delta_ik W_j - delta_jk W_i
so the [96,96,96,3,3,3] output is +-W_c scattered over 27 slots per point.

Sharding: axis 0 (12 planes per core x 8 cores). h is analytic in the
inputs, so each core evaluates its slab plus a 1-plane halo directly --
no inter-core exchange. Per core the grid is row-packed: row = a0*96+a1
(1152 rows -> 9 tiles of 128 partitions), free dim = a2 (96); h lives on
an 11-tile extended row window (halo tiles at both ends).

This version is built to hide all compute under the output-write DMA
(11.9 MB/core, the memory roofline):
- All runtime scalars/profiles (mass halves, mass ratio, per-row xy
  distance^2 `ab`, z profile `crow`) are computed on the host and shipped
  as one small `misc` input, so the device h-field pipeline is just:
  r_n = Sqrt(crow_n + ab_n) (fused activation bias), q_n = 1/r_n,
  psi-1 = mh1*q1 + mh2*q2 (fused STT + activation scale), hsq = psi^2,
  h(bf16) = hsq^2.
- h is kept in a single bf16 copy (tolerance 2e-2 >> bf16 FD error);
  axis-0/1 derivatives are one 3-term matmul accumulation each against
  host-built band matrices with exact-bf16 +-1/+-2 entries (the
  0.5/(2DX) Christoffel/FD factor is folded into hc = (0.25/DX)/h).
- axis-2 derivative via forward diffs d[z]=h[z+1]-h[z]; interior central
  diff = d[z]+d[z-1], edges = 2*d -> uniform scale, folded into hc too.
- The 27-slot scatter writes the 9 diagonal slots fused with the W
  multiply (stride-0 broadcast sources), the 12 off-diagonal slots as 6
  paired-slot copies; scatter work is spread across Vector/GpSimd/Scalar.
- Emission interleaves h chunks with per-tile work so tile 0's output
  DMA launches within a few us and the DMA stays saturated.
"""

import numpy as np

RES = 96
N_CORES = 8
PLANES = RES // N_CORES        # 12
LROWS = PLANES * RES           # 1152 local rows
NT = LROWS // 128              # 9 local 128-row tiles
EXTNT = NT + 2                 # 11 extended tiles (halo)
NROWS_G = RES * RES            # 9216 global rows
S27 = 27
NOB = 4                        # rotating output buffers
HW_ = EXTNT * RES              # 1056 ext free width

# misc input layout (fp32 columns, identical on all 128 partitions except ab)
M_CROW1 = 0      # (z - pz1)^2 [96]
M_CROW2 = 96     # (z - pz2)^2 [96]
M_AB1 = 192      # (x-px1)^2+(y-py1)^2 per ext block [11]
M_AB2 = 203      # [11]
M_MH2 = 214      # m2/2
M_K = 215        # (m1/2)/(m2/2)
MISCW = 216


def _grid_x():
    # Match the reference grid bit-for-bit: jnp.linspace in fp32 on CPU
    # (the reference's softplus cannot compile for the neuron backend, so
    # it necessarily runs on the jax CPU platform).
    import jax
    import jax.numpy as jnp
    MAX_X = 1.0
    DX = np.float32(MAX_X / (RES / 2 - 1))

    def _ls():
        return jnp.linspace(
            DX * (1 - RES / 2), DX * (RES / 2 - 1), RES, dtype=jnp.float32
        )

    try:
        with jax.default_device(jax.devices("cpu")[0]):
            x = np.asarray(_ls())
    except Exception:
        x = np.asarray(_ls())
    return x, float(DX)


def _fd_sources(idx, coeff_c, coeff_e):
    """(offset, coeff) pairs for d/didx with 1st-order one-sided edges."""
    if idx == 0:
        return [(1, coeff_e), (0, -coeff_e)]
    if idx == RES - 1:
        return [(0, coeff_e), (-1, -coeff_e)]
    return [(1, coeff_c), (-1, -coeff_c)]


def _build_dmat(core):
    """[128, 6*3*128] bf16 FD matrices as matmul lhsT ([q, p] = coeff of
    ext-row q in output row p). The 0.5/(2DX) factor lives in hc, so
    entries are +-1 (interior) / +-2 (grid edge), exact in bf16.
    Entries: 0 g0(t=0), 1 g0(interior), 2 g0(t=8), 3..5 g1(t%3)."""
    import ml_dtypes
    out = np.zeros((128, 6 * 3 * 128), np.float64)

    def fill(entry, t, axis):
        for p in range(128):
            gr = core * LROWS + 128 * t + p
            a = (gr // RES) if axis == 0 else (gr % RES)
            step = RES if axis == 0 else 1
            for off, cf in _fd_sources(a, 1.0, 2.0):
                g2 = gr + off * step
                e_ = g2 - core * LROWS + 128
                j = e_ // 128 - t
                q = e_ - 128 * (t + j)
                assert 0 <= j <= 2 and 0 <= q < 128, (core, t, p, off)
                out[q, (entry * 3 + j) * 128 + p] = cf

    fill(0, 0, 0)
    fill(1, 1, 0)
    fill(2, NT - 1, 0)
    for v in range(3):
        fill(3 + v, v, 1)
    return out.astype(ml_dtypes.bfloat16)


def _core_xy(core, x):
    """Per-ext-row (x, y) grid coordinates, halo overrun clamped."""
    slab = core * LROWS
    e = np.arange(EXTNT * 128)
    g = np.clip(slab - 128 + e, 0, NROWS_G - 1)
    xcol = x[g % RES].reshape(EXTNT, 128).T      # X coordinate (a1)
    ycol = x[g // RES].reshape(EXTNT, 128).T     # Y coordinate (a0)
    return xcol.astype(np.float64), ycol.astype(np.float64)


def _build_program(DX):
    import dataclasses as _dc

    import concourse.bacc as bacc
    import concourse.mybir as mybir
    import concourse.tile as tile
    from concourse.alu_op_type import AluOpType

    DT = mybir.dt.float32
    BF = mybir.dt.bfloat16
    AF = mybir.ActivationFunctionType
    SQC = float(np.sqrt(0.25 / np.float64(DX)))   # hc = (SQC/hsq)^2

    nc = bacc.Bacc(None, target_bir_lowering=False, debug=True)
    d_misc = nc.dram_tensor("misc", [128, MISCW], DT, kind="ExternalInput")
    d_dmat = nc.dram_tensor("dmat", [128, 6 * 3 * 128], BF, kind="ExternalInput")
    d_out = nc.dram_tensor("out", [LROWS, RES * S27], DT, kind="ExternalOutput")

    # ext-block chunks and the local tiles each unlocks (tile t reads ext
    # blocks t..t+2, hsq block t+1, d block t)
    CHUNKS = [(range(0, 4), range(0, 2)),
              (range(4, 8), range(2, 6)),
              (range(8, 11), range(6, 9))]

    with tile.TileContext(nc) as tc:
        with (
            tc.tile_pool(name="const", bufs=1) as cpool,
            tc.tile_pool(name="work", bufs=3) as wpool,
            tc.tile_pool(name="wout", bufs=4) as wopool,
            tc.tile_pool(name="obuf", bufs=NOB) as opool,
            tc.tile_pool(name="psum", bufs=4, space="PSUM") as pspool,
        ):
            mi = cpool.tile([128, MISCW], DT)
            nc.sync.dma_start(mi[:], d_misc[:])
            dm = cpool.tile([128, 6 * 3 * 128], BF)
            nc.sync.dma_start(dm[:], d_dmat[:])

            HSQ = cpool.tile([128, HW_], DT)          # psi^2 (h = HSQ^2)
            HB = cpool.tile([128, HW_], BF)           # h in bf16
            HB3 = HB[:].rearrange("p (b z) -> p b z", z=RES)
            D = cpool.tile([128, NT * RES], DT)       # fwd z-diffs of h
            D3 = D[:].rearrange("p (t z) -> p t z", z=RES)

            # rotating output buffers, zero slots pre-filled once
            otiles = []
            for i in range(NOB):
                O = opool.tile([128, RES * S27], DT, tag=f"ob{i}")
                O3 = O[:].rearrange("p (z s) -> p z s", s=S27)
                eng = (nc.vector, nc.gpsimd, nc.vector, nc.gpsimd)[i]
                eng.memset(O3[:, :, 5:8:2], 0.0)
                eng.memset(O3[:, :, 11:20:4], 0.0)
                eng.memset(O3[:, :, 21], 0.0)
                otiles.append((O, O3))

            mh2 = mi[:, M_MH2:M_MH2 + 1]
            kcol = mi[:, M_K:M_K + 1]
            crow1 = mi[:, M_CROW1:M_CROW1 + RES]
            crow2 = mi[:, M_CROW2:M_CROW2 + RES]

            def emit_chunk(ci):
                blocks, tiles_d = CHUNKS[ci]
                b0, bn = blocks[0], len(blocks)
                csl = slice(RES * b0, RES * (b0 + bn))
                R1 = wpool.tile([128, bn * RES], DT, tag="r1")
                R2 = wpool.tile([128, bn * RES], DT, tag="r2")
                for e in blocks:
                    o = (e - b0) * RES
                    nc.scalar.activation(R1[:, o:o + RES], crow1, AF.Sqrt,
                                         bias=mi[:, M_AB1 + e:M_AB1 + e + 1])
                    nc.scalar.activation(R2[:, o:o + RES], crow2, AF.Sqrt,
                                         bias=mi[:, M_AB2 + e:M_AB2 + e + 1])
                Q1 = wpool.tile([128, bn * RES], DT, tag="q1")
                nc.vector.reciprocal_approx_fast(Q1[:], R1[:])
                Q2 = wpool.tile([128, bn * RES], DT, tag="q2")
                nc.vector.reciprocal_approx_fast(Q2[:], R2[:])
                P = wpool.tile([128, bn * RES], DT, tag="pp")
                nc.vector.scalar_tensor_tensor(
                    P[:], Q1[:], kcol, Q2[:], AluOpType.mult, AluOpType.add
                )
                # hsq = (mh2*(k*q1+q2) + 1)^2 = psi^2 ; h = hsq^2 (bf16)
                nc.scalar.activation(HSQ[:, csl], P[:], AF.Square,
                                     bias=1.0, scale=mh2)
                nc.scalar.activation(HB[:, csl], HSQ[:, csl], AF.Square)
                # forward z-diffs for the local tiles this chunk covers
                ta, tb = tiles_d[0], tiles_d[-1]
                nc.gpsimd.tensor_sub(
                    D3[:, ta:tb + 1, 0:RES - 1],
                    HB3[:, ta + 1:tb + 2, 1:RES],
                    HB3[:, ta + 1:tb + 2, 0:RES - 1],
                )

            def emit_tile(t):
                g0e = 0 if t == 0 else (2 if t == NT - 1 else 1)
                g1e = 3 + (t % 3)
                p0 = pspool.tile([128, RES], DT, tag="p0")
                p1 = pspool.tile([128, RES], DT, tag="p1")
                for ge, pp in ((g0e, p0), (g1e, p1)):
                    for j in range(3):
                        lhs = dm[:, (ge * 3 + j) * 128:(ge * 3 + j + 1) * 128]
                        rsl = slice(RES * (t + j), RES * (t + j + 1))
                        nc.tensor.matmul(pp[:], lhs, HB[:, rsl],
                                         start=(j == 0), stop=(j == 2))

                hsl = slice(RES * (t + 1), RES * (t + 2))
                vinv = wopool.tile([128, RES], DT, tag="vinv")
                nc.vector.reciprocal_approx_fast(vinv[:], HSQ[:, hsl])
                hc = wopool.tile([128, RES], DT, tag="hc")
                nc.scalar.activation(hc[:], vinv[:], AF.Square, scale=SQC)

                # z central diffs from forward diffs (uniform hc scale)
                ST = wopool.tile([128, RES], DT, tag="st")
                nc.gpsimd.tensor_add(ST[:, 1:95], D3[:, t, 1:95], D3[:, t, 0:94])
                nc.gpsimd.tensor_scalar_mul(
                    ST[:, 0:RES:RES - 1], D[:, RES * t:RES * t + 95:94], 2.0
                )

                O, O3 = otiles[t % NOB]
                # W_c into the i=0 diagonal slots (dense strided writes;
                # PSUM sources must stay off GpSimd)
                nc.vector.tensor_mul(O3[:, :, 0], p0[:], hc[:])
                nc.vector.tensor_mul(O3[:, :, 1], p1[:], hc[:])
                nc.gpsimd.tensor_mul(O3[:, :, 2], ST[:], hc[:])
                # replicate to the i=1,2 diagonal slots (plain copies)
                nc.vector.tensor_copy(O3[:, :, 12:15], O3[:, :, 0:3])
                nc.gpsimd.tensor_copy(O3[:, :, 24:27], O3[:, :, 0:3])

                # off-diagonal pairs, sources = the diag slots c=0,1,2
                def src(c):
                    ap = O3[:, :, c]
                    return _dc.replace(ap, ap=ap.ap + [[0, 2]])
                nc.scalar.copy(O3[:, :, 10:21:10], src(0))        # +W0
                nc.scalar.mul(O3[:, :, 4:9:4], src(0), -1.0)      # -W0
                nc.scalar.copy(O3[:, :, 3:24:20], src(1))         # +W1
                nc.vector.tensor_scalar_mul(O3[:, :, 9:18:8], src(1), -1.0)
                nc.gpsimd.tensor_copy(O3[:, :, 6:17:10], src(2))  # +W2
                nc.gpsimd.tensor_scalar_mul(O3[:, :, 18:23:4], src(2), -1.0)

                nc.sync.dma_start(d_out[128 * t:128 * (t + 1), :], O[:])

            emit_chunk(0)
            emit_tile(0)
            emit_tile(1)
            emit_chunk(1)
            for t in range(2, 6):
                emit_tile(t)
            emit_chunk(2)
            for t in range(6, NT):
                emit_tile(t)

    nc.finalize()
    return nc


_CACHE = {}


def _get_setup():
    if "nc" not in _CACHE:
        x, DX = _grid_x()
        _CACHE["x"] = x
        _CACHE["dmat"] = [_build_dmat(c) for c in range(N_CORES)]
        _CACHE["xy"] = [_core_xy(c, x) for c in range(N_CORES)]
        _CACHE["nc"] = _build_program(DX)
    return _CACHE


def _in_maps(BH_positions, BH_masses_presoftplus):
    cache = _get_setup()
    x = cache["x"].astype(np.float64)
    pos = np.asarray(BH_positions, np.float32).astype(np.float64)
    pre = np.asarray(BH_masses_presoftplus, np.float32)
    masses = np.log1p(np.exp(pre)).astype(np.float32).astype(np.float64)
    mh = 0.5 * masses
    crow = [(x - pos[n, 2]) ** 2 for n in range(2)]

    maps = []
    for c in range(N_CORES):
        xcol, ycol = cache["xy"][c]
        misc = np.zeros((128, MISCW), np.float32)
        misc[:, M_CROW1:M_CROW1 + RES] = crow[0][None, :]
        misc[:, M_CROW2:M_CROW2 + RES] = crow[1][None, :]
        misc[:, M_AB1:M_AB1 + EXTNT] = (
            (xcol - pos[0, 0]) ** 2 + (ycol - pos[0, 1]) ** 2
        )
        misc[:, M_AB2:M_AB2 + EXTNT] = (
            (xcol - pos[1, 0]) ** 2 + (ycol - pos[1, 1]) ** 2
        )
        misc[:, M_MH2] = mh[1]
        misc[:, M_K] = mh[0] / mh[1]
        maps.append({"misc": misc, "dmat": cache["dmat"][c]})
    return cache["nc"], maps


def kernel(BH_positions, BH_masses_presoftplus):
    from concourse.bass_utils import run_bass_kernel_spmd

    nc, in_maps = _in_maps(BH_positions, BH_masses_presoftplus)
    res = run_bass_kernel_spmd(nc, in_maps, list(range(N_CORES)))
    parts = [
        res.results[c]["out"].reshape(PLANES, RES, RES, 3, 3, 3)
        for c in range(N_CORES)
    ]
    return np.ascontiguousarray(np.concatenate(parts, axis=0))


# revision 10
# speedup vs baseline: 1.0235x; 1.0235x over previous
"""Trainium2 Bass kernel for the Brill-Lindquist Christoffel-symbol grid.

Math: the reference reduces to
    psi  = 1 + sum_n m_n / (2 r_n),   m = softplus(pre)
    h    = psi^4
    G_c  = finite-difference gradient of h along grid axis c (2nd order
           central interior, 1st order one-sided edges, spacing DX)
    W_c  = 0.5 * G_c / h
    Gamma^i_{jk} = delta_ij W_k + delta_ik W_j - delta_jk W_i
so the [96,96,96,3,3,3] output is +-W_c scattered over 27 slots per point.

Sharding: axis 0 (12 planes per core x 8 cores). h is analytic in the
inputs, so each core evaluates its slab plus a 1-plane halo directly --
no inter-core exchange. Per core the grid is row-packed: row = a0*96+a1
(1152 rows -> 9 tiles of 128 partitions), free dim = a2 (96); h lives on
an 11-tile extended row window (halo tiles at both ends).

This version is built to hide all compute under the output-write DMA
(11.9 MB/core, the memory roofline):
- All runtime scalars/profiles (mass halves, mass ratio, per-row xy
  distance^2 `ab`, z profile `crow`) are computed on the host and shipped
  as one small `misc` input, so the device h-field pipeline is just:
  r_n = Sqrt(crow_n + ab_n) (fused activation bias), q_n = 1/r_n,
  psi-1 = mh1*q1 + mh2*q2 (fused STT + activation scale), hsq = psi^2,
  h(bf16) = hsq^2.
- h is kept in a single bf16 copy (tolerance 2e-2 >> bf16 FD error);
  axis-0/1 derivatives are one 3-term matmul accumulation each against
  host-built band matrices with exact-bf16 +-1/+-2 entries (the
  0.5/(2DX) Christoffel/FD factor is folded into hc = (0.25/DX)/h).
- axis-2 derivative via forward diffs d[z]=h[z+1]-h[z]; interior central
  diff = d[z]+d[z-1], edges = 2*d -> uniform scale, folded into hc too.
- The 27-slot scatter writes the 9 diagonal slots fused with the W
  multiply (stride-0 broadcast sources), the 12 off-diagonal slots as 6
  paired-slot copies; scatter work is spread across Vector/GpSimd/Scalar.
- Emission interleaves h chunks with per-tile work so tile 0's output
  DMA launches within a few us and the DMA stays saturated.
"""

import numpy as np

RES = 96
N_CORES = 8
PLANES = RES // N_CORES        # 12
LROWS = PLANES * RES           # 1152 local rows
NT = LROWS // 128              # 9 local 128-row tiles
EXTNT = NT + 2                 # 11 extended tiles (halo)
NROWS_G = RES * RES            # 9216 global rows
S27 = 27
NOB = 4                        # rotating output buffers
HW_ = EXTNT * RES              # 1056 ext free width

# misc input layout (fp32 columns, identical on all 128 partitions except ab)
M_CROW1 = 0      # (z - pz1)^2 [96]
M_CROW2 = 96     # (z - pz2)^2 [96]
M_AB1 = 192      # (x-px1)^2+(y-py1)^2 per ext block [11]
M_AB2 = 203      # [11]
M_MH1 = 214      # m1/2
M_MH2 = 215      # m2/2
MISCW = 216


def _grid_x():
    # Match the reference grid bit-for-bit: jnp.linspace in fp32 on CPU
    # (the reference's softplus cannot compile for the neuron backend, so
    # it necessarily runs on the jax CPU platform).
    import jax
    import jax.numpy as jnp
    MAX_X = 1.0
    DX = np.float32(MAX_X / (RES / 2 - 1))

    def _ls():
        return jnp.linspace(
            DX * (1 - RES / 2), DX * (RES / 2 - 1), RES, dtype=jnp.float32
        )

    try:
        with jax.default_device(jax.devices("cpu")[0]):
            x = np.asarray(_ls())
    except Exception:
        x = np.asarray(_ls())
    return x, float(DX)


def _fd_sources(idx, coeff_c, coeff_e):
    """(offset, coeff) pairs for d/didx with 1st-order one-sided edges."""
    if idx == 0:
        return [(1, coeff_e), (0, -coeff_e)]
    if idx == RES - 1:
        return [(0, coeff_e), (-1, -coeff_e)]
    return [(1, coeff_c), (-1, -coeff_c)]


def _build_dmat(core):
    """[128, 6*3*128] bf16 FD matrices as matmul lhsT ([q, p] = coeff of
    ext-row q in output row p). The 0.5/(2DX) factor lives in hc, so
    entries are +-1 (interior) / +-2 (grid edge), exact in bf16.
    Entries: 0 g0(t=0), 1 g0(interior), 2 g0(t=8), 3..5 g1(t%3)."""
    import ml_dtypes
    out = np.zeros((128, 6 * 3 * 128), np.float64)

    def fill(entry, t, axis):
        for p in range(128):
            gr = core * LROWS + 128 * t + p
            a = (gr // RES) if axis == 0 else (gr % RES)
            step = RES if axis == 0 else 1
            for off, cf in _fd_sources(a, 1.0, 2.0):
                g2 = gr + off * step
                e_ = g2 - core * LROWS + 128
                j = e_ // 128 - t
                q = e_ - 128 * (t + j)
                assert 0 <= j <= 2 and 0 <= q < 128, (core, t, p, off)
                out[q, (entry * 3 + j) * 128 + p] = cf

    fill(0, 0, 0)
    fill(1, 1, 0)
    fill(2, NT - 1, 0)
    for v in range(3):
        fill(3 + v, v, 1)
    return out.astype(ml_dtypes.bfloat16)


def _core_xy(core, x):
    """Per-ext-row (x, y) grid coordinates, halo overrun clamped."""
    slab = core * LROWS
    e = np.arange(EXTNT * 128)
    g = np.clip(slab - 128 + e, 0, NROWS_G - 1)
    xcol = x[g % RES].reshape(EXTNT, 128).T      # X coordinate (a1)
    ycol = x[g // RES].reshape(EXTNT, 128).T     # Y coordinate (a0)
    return xcol.astype(np.float64), ycol.astype(np.float64)


def _build_program(DX):
    import dataclasses as _dc

    import concourse.bacc as bacc
    import concourse.mybir as mybir
    import concourse.tile as tile
    from concourse.alu_op_type import AluOpType

    DT = mybir.dt.float32
    BF = mybir.dt.bfloat16
    AF = mybir.ActivationFunctionType
    SQC = float(np.sqrt(0.25 / np.float64(DX)))   # hc = (SQC/hsq)^2

    nc = bacc.Bacc(None, target_bir_lowering=False, debug=True)
    d_misc = nc.dram_tensor("misc", [128, MISCW], DT, kind="ExternalInput")
    d_dmat = nc.dram_tensor("dmat", [128, 6 * 3 * 128], BF, kind="ExternalInput")
    d_out = nc.dram_tensor("out", [LROWS, RES * S27], DT, kind="ExternalOutput")

    # ext-block chunks and the local tiles each unlocks (tile t reads ext
    # blocks t..t+2, hsq block t+1, d block t)
    CHUNKS = [(range(0, 4), range(0, 2)),
              (range(4, 8), range(2, 6)),
              (range(8, 11), range(6, 9))]

    with tile.TileContext(nc) as tc:
        with (
            tc.tile_pool(name="const", bufs=1) as cpool,
            tc.tile_pool(name="work", bufs=3) as wpool,
            tc.tile_pool(name="wout", bufs=4) as wopool,
            tc.tile_pool(name="obuf", bufs=1) as opool,
            tc.tile_pool(name="psum", bufs=4, space="PSUM") as pspool,
        ):
            mi = cpool.tile([128, MISCW], DT)
            nc.sync.dma_start(mi[:], d_misc[:])
            dm = cpool.tile([128, 6 * 3 * 128], BF)
            nc.sync.dma_start(dm[:], d_dmat[:])

            HSQ = cpool.tile([128, HW_], DT)          # psi^2 (h = HSQ^2)
            HB = cpool.tile([128, HW_], BF)           # h in bf16
            HB3 = HB[:].rearrange("p (b z) -> p b z", z=RES)
            D = cpool.tile([128, NT * RES], DT)       # fwd z-diffs of h
            D3 = D[:].rearrange("p (t z) -> p t z", z=RES)

            # rotating output buffers (slot-major: free = s*96+z), zero
            # slots {5,7,11,15,19,21} pre-filled once, never rewritten
            otiles = []
            for i in range(NOB):
                O = opool.tile([128, RES * S27], DT, tag=f"ob{i}")
                OS = O[:].rearrange("p (s z) -> p s z", z=RES)
                eng = (nc.vector, nc.gpsimd, nc.vector, nc.gpsimd)[i]
                eng.memset(OS[:, 5:8:2, :], 0.0)
                eng.memset(OS[:, 11:20:4, :], 0.0)
                eng.memset(OS[:, 21, :], 0.0)
                otiles.append((O, OS))

            mh1 = mi[:, M_MH1:M_MH1 + 1]
            mh2 = mi[:, M_MH2:M_MH2 + 1]
            crow1 = mi[:, M_CROW1:M_CROW1 + RES]
            crow2 = mi[:, M_CROW2:M_CROW2 + RES]

            def emit_chunk(ci):
                blocks, tiles_d = CHUNKS[ci]
                b0, bn = blocks[0], len(blocks)
                csl = slice(RES * b0, RES * (b0 + bn))
                R1 = wpool.tile([128, bn * RES], DT, tag="r1")
                R2 = wpool.tile([128, bn * RES], DT, tag="r2")
                for e in blocks:
                    o = (e - b0) * RES
                    nc.scalar.activation(R1[:, o:o + RES], crow1, AF.Sqrt,
                                         bias=mi[:, M_AB1 + e:M_AB1 + e + 1])
                    nc.scalar.activation(R2[:, o:o + RES], crow2, AF.Sqrt,
                                         bias=mi[:, M_AB2 + e:M_AB2 + e + 1])
                Q1 = wpool.tile([128, bn * RES], DT, tag="q1")
                nc.vector.reciprocal_approx_fast(Q1[:], R1[:])
                Q2 = wpool.tile([128, bn * RES], DT, tag="q2")
                nc.vector.reciprocal_approx_fast(Q2[:], R2[:])
                # psi = mh1*q1 + mh2*q2 + 1 ; hsq = psi^2 ; h = hsq^2 (bf16)
                A = wpool.tile([128, bn * RES], DT, tag="aa")
                nc.gpsimd.tensor_scalar(A[:], Q1[:], mh1, 1.0,
                                        AluOpType.mult, AluOpType.add)
                B = wpool.tile([128, bn * RES], DT, tag="bb")
                nc.vector.tensor_scalar(B[:], Q2[:], mh2, None, AluOpType.mult)
                PSI = wpool.tile([128, bn * RES], DT, tag="psi")
                nc.gpsimd.tensor_add(PSI[:], A[:], B[:])
                nc.vector.tensor_mul(HSQ[:, csl], PSI[:], PSI[:])
                nc.gpsimd.tensor_mul(HB[:, csl], HSQ[:, csl], HSQ[:, csl])
                # forward z-diffs for the local tiles this chunk covers
                ta, tb = tiles_d[0], tiles_d[-1]
                nc.gpsimd.tensor_sub(
                    D3[:, ta:tb + 1, 0:RES - 1],
                    HB3[:, ta + 1:tb + 2, 1:RES],
                    HB3[:, ta + 1:tb + 2, 0:RES - 1],
                )

            def emit_tile(t):
                g0e = 0 if t == 0 else (2 if t == NT - 1 else 1)
                g1e = 3 + (t % 3)
                p0 = pspool.tile([128, RES], DT, tag="p0")
                p1 = pspool.tile([128, RES], DT, tag="p1")
                for ge, pp in ((g0e, p0), (g1e, p1)):
                    for j in range(3):
                        lhs = dm[:, (ge * 3 + j) * 128:(ge * 3 + j + 1) * 128]
                        rsl = slice(RES * (t + j), RES * (t + j + 1))
                        nc.tensor.matmul(pp[:], lhs, HB[:, rsl],
                                         start=(j == 0), stop=(j == 2))

                hsl = slice(RES * (t + 1), RES * (t + 2))
                vinv = wopool.tile([128, RES], DT, tag="vinv")
                nc.vector.reciprocal_approx_fast(vinv[:], HSQ[:, hsl])
                hc = wopool.tile([128, RES], DT, tag="hc")
                nc.scalar.activation(hc[:], vinv[:], AF.Square, scale=SQC)

                # z central diffs from forward diffs (uniform hc scale)
                ST = wopool.tile([128, RES], DT, tag="st")
                nc.gpsimd.tensor_add(ST[:, 1:95], D3[:, t, 1:95], D3[:, t, 0:94])
                nc.gpsimd.tensor_scalar_mul(
                    ST[:, 0:RES:RES - 1], D[:, RES * t:RES * t + 95:94], 2.0
                )

                O, OS = otiles[t % NOB]
                # slot-major output: every write below is a dense 96-run.
                # W_c into slots 0..2 (PSUM sources must stay off GpSimd)
                nc.vector.tensor_mul(OS[:, 0, :], p0[:], hc[:])
                nc.vector.tensor_mul(OS[:, 1, :], p1[:], hc[:])
                nc.gpsimd.tensor_mul(OS[:, 2, :], ST[:], hc[:])
                # diagonal i=1,2 blocks (slots 12-14, 24-26) in one copy
                dap = O[:, 1152:1440]
                ddst = _dc.replace(dap, ap=[dap.ap[0], [1152, 2], [1, 288]])
                sap = O[:, 0:288]
                dsrc = _dc.replace(sap, ap=[sap.ap[0], [0, 2], [1, 288]])
                nc.vector.tensor_copy(ddst, dsrc)

                # off-diagonal slot pairs, sources = slots 0..2
                def src2(c):
                    ap = OS[:, c, :]
                    return _dc.replace(ap, ap=[ap.ap[0], [0, 2]] + ap.ap[1:])
                nc.scalar.copy(OS[:, 10:21:10, :], src2(0))         # +W0
                nc.scalar.mul(OS[:, 4:9:4, :], src2(0), -1.0)       # -W0
                nc.scalar.copy(OS[:, 3:24:20, :], src2(1))          # +W1
                nc.scalar.mul(OS[:, 9:18:8, :], src2(1), -1.0)      # -W1
                nc.gpsimd.tensor_copy(OS[:, 6:17:10, :], src2(2))   # +W2
                nc.gpsimd.tensor_scalar_mul(OS[:, 18:23:4, :], src2(2), -1.0)

                nc.sync.dma_start(d_out[128 * t:128 * (t + 1), :], O[:])

            emit_chunk(0)
            emit_tile(0)
            emit_tile(1)
            emit_chunk(1)
            for t in range(2, 6):
                emit_tile(t)
            emit_chunk(2)
            for t in range(6, NT):
                emit_tile(t)

    nc.finalize()
    return nc


_CACHE = {}


def _get_setup():
    if "nc" not in _CACHE:
        x, DX = _grid_x()
        _CACHE["x"] = x
        _CACHE["dmat"] = [_build_dmat(c) for c in range(N_CORES)]
        _CACHE["xy"] = [_core_xy(c, x) for c in range(N_CORES)]
        _CACHE["nc"] = _build_program(DX)
    return _CACHE


def _in_maps(BH_positions, BH_masses_presoftplus):
    cache = _get_setup()
    x = cache["x"].astype(np.float64)
    pos = np.asarray(BH_positions, np.float32).astype(np.float64)
    pre = np.asarray(BH_masses_presoftplus, np.float32)
    masses = np.log1p(np.exp(pre)).astype(np.float32).astype(np.float64)
    mh = 0.5 * masses
    crow = [(x - pos[n, 2]) ** 2 for n in range(2)]

    maps = []
    for c in range(N_CORES):
        xcol, ycol = cache["xy"][c]
        misc = np.zeros((128, MISCW), np.float32)
        misc[:, M_CROW1:M_CROW1 + RES] = crow[0][None, :]
        misc[:, M_CROW2:M_CROW2 + RES] = crow[1][None, :]
        misc[:, M_AB1:M_AB1 + EXTNT] = (
            (xcol - pos[0, 0]) ** 2 + (ycol - pos[0, 1]) ** 2
        )
        misc[:, M_AB2:M_AB2 + EXTNT] = (
            (xcol - pos[1, 0]) ** 2 + (ycol - pos[1, 1]) ** 2
        )
        misc[:, M_MH1] = mh[0]
        misc[:, M_MH2] = mh[1]
        maps.append({"misc": misc, "dmat": cache["dmat"][c]})
    return cache["nc"], maps


def kernel(BH_positions, BH_masses_presoftplus):
    from concourse.bass_utils import run_bass_kernel_spmd

    nc, in_maps = _in_maps(BH_positions, BH_masses_presoftplus)
    res = run_bass_kernel_spmd(nc, in_maps, list(range(N_CORES)))
    # device rows are [a0, a1] x slot-major free (s*96 + a2); permute on host
    out = np.empty((RES, RES, RES, 3, 3, 3), np.float32)
    ov = out.reshape(N_CORES, PLANES, RES, RES, S27)
    for c in range(N_CORES):
        part = res.results[c]["out"].reshape(PLANES, RES, S27, RES)
        ov[c] = part.transpose(0, 1, 3, 2)
    return out


# revision 12
# speedup vs baseline: 1.4446x; 1.4114x over previous
"""Trainium2 Bass kernel for the Brill-Lindquist Christoffel-symbol grid.

Math: the reference reduces to
    psi  = 1 + sum_n m_n / (2 r_n),   m = softplus(pre)
    h    = psi^4
    G_c  = finite-difference gradient of h along grid axis c (2nd order
           central interior, 1st order one-sided edges, spacing DX)
    W_c  = 0.5 * G_c / h
    Gamma^i_{jk} = delta_ij W_k + delta_ik W_j - delta_jk W_i
so the [96,96,96,3,3,3] output is +-W_c scattered over 27 slots per point.

Sharding: axis 0 (12 planes per core x 8 cores). h is analytic in the
inputs, so each core evaluates its slab plus a 1-plane halo directly --
no inter-core exchange. Per core the grid is row-packed: row = a0*96+a1
(1152 rows -> 9 tiles of 128 partitions), free dim = a2 (96); h lives on
an 11-tile extended row window (halo tiles at both ends).

This version is built to hide all compute under the output-write DMA
(11.9 MB/core, the memory roofline):
- All runtime scalars/profiles (mass halves, mass ratio, per-row xy
  distance^2 `ab`, z profile `crow`) are computed on the host and shipped
  as one small `misc` input, so the device h-field pipeline is just:
  r_n = Sqrt(crow_n + ab_n) (fused activation bias), q_n = 1/r_n,
  psi-1 = mh1*q1 + mh2*q2 (fused STT + activation scale), hsq = psi^2,
  h(bf16) = hsq^2.
- h is kept in a single bf16 copy (tolerance 2e-2 >> bf16 FD error);
  axis-0/1 derivatives are one 3-term matmul accumulation each against
  host-built band matrices with exact-bf16 +-1/+-2 entries (the
  0.5/(2DX) Christoffel/FD factor is folded into hc = (0.25/DX)/h).
- axis-2 derivative via forward diffs d[z]=h[z+1]-h[z]; interior central
  diff = d[z]+d[z-1], edges = 2*d -> uniform scale, folded into hc too.
- The 27-slot scatter writes the 9 diagonal slots fused with the W
  multiply (stride-0 broadcast sources), the 12 off-diagonal slots as 6
  paired-slot copies; scatter work is spread across Vector/GpSimd/Scalar.
- Emission interleaves h chunks with per-tile work so tile 0's output
  DMA launches within a few us and the DMA stays saturated.
"""

import numpy as np

RES = 96
N_CORES = 8
PLANES = RES // N_CORES        # 12
LROWS = PLANES * RES           # 1152 local rows
NT = LROWS // 128              # 9 local 128-row tiles
EXTNT = NT + 2                 # 11 extended tiles (halo)
NROWS_G = RES * RES            # 9216 global rows
S27 = 27
NOB = 4                        # rotating output buffers
HW_ = EXTNT * RES              # 1056 ext free width

# misc input layout (fp32 columns, identical on all 128 partitions except ab)
M_CROW1 = 0      # (z - pz1)^2 [96]
M_CROW2 = 96     # (z - pz2)^2 [96]
M_AB1 = 192      # (x-px1)^2+(y-py1)^2 per ext block [11]
M_AB2 = 203      # [11]
M_MH1 = 214      # m1/2
M_MH2 = 215      # m2/2
MISCW = 216


def _grid_x():
    # Match the reference grid bit-for-bit: jnp.linspace in fp32 on CPU
    # (the reference's softplus cannot compile for the neuron backend, so
    # it necessarily runs on the jax CPU platform).
    import jax
    import jax.numpy as jnp
    MAX_X = 1.0
    DX = np.float32(MAX_X / (RES / 2 - 1))

    def _ls():
        return jnp.linspace(
            DX * (1 - RES / 2), DX * (RES / 2 - 1), RES, dtype=jnp.float32
        )

    try:
        with jax.default_device(jax.devices("cpu")[0]):
            x = np.asarray(_ls())
    except Exception:
        x = np.asarray(_ls())
    return x, float(DX)


def _fd_sources(idx, coeff_c, coeff_e):
    """(offset, coeff) pairs for d/didx with 1st-order one-sided edges."""
    if idx == 0:
        return [(1, coeff_e), (0, -coeff_e)]
    if idx == RES - 1:
        return [(0, coeff_e), (-1, -coeff_e)]
    return [(1, coeff_c), (-1, -coeff_c)]


def _build_dmat(core):
    """[128, 6*3*128] bf16 FD matrices as matmul lhsT ([q, p] = coeff of
    ext-row q in output row p). The 0.5/(2DX) factor lives in hc, so
    entries are +-1 (interior) / +-2 (grid edge), exact in bf16.
    Entries: 0 g0(t=0), 1 g0(interior), 2 g0(t=8), 3..5 g1(t%3)."""
    import ml_dtypes
    out = np.zeros((128, 6 * 3 * 128), np.float64)

    def fill(entry, t, axis):
        for p in range(128):
            gr = core * LROWS + 128 * t + p
            a = (gr // RES) if axis == 0 else (gr % RES)
            step = RES if axis == 0 else 1
            for off, cf in _fd_sources(a, 1.0, 2.0):
                g2 = gr + off * step
                e_ = g2 - core * LROWS + 128
                j = e_ // 128 - t
                q = e_ - 128 * (t + j)
                assert 0 <= j <= 2 and 0 <= q < 128, (core, t, p, off)
                out[q, (entry * 3 + j) * 128 + p] = cf

    fill(0, 0, 0)
    fill(1, 1, 0)
    fill(2, NT - 1, 0)
    for v in range(3):
        fill(3 + v, v, 1)
    return out.astype(ml_dtypes.bfloat16)


def _core_xy(core, x):
    """Per-ext-row (x, y) grid coordinates, halo overrun clamped."""
    slab = core * LROWS
    e = np.arange(EXTNT * 128)
    g = np.clip(slab - 128 + e, 0, NROWS_G - 1)
    xcol = x[g % RES].reshape(EXTNT, 128).T      # X coordinate (a1)
    ycol = x[g // RES].reshape(EXTNT, 128).T     # Y coordinate (a0)
    return xcol.astype(np.float64), ycol.astype(np.float64)


def _build_program(DX):
    import dataclasses as _dc

    import concourse.bacc as bacc
    import concourse.mybir as mybir
    import concourse.tile as tile
    from concourse.alu_op_type import AluOpType

    DT = mybir.dt.float32
    BF = mybir.dt.bfloat16
    AF = mybir.ActivationFunctionType
    SQC = float(np.sqrt(0.25 / np.float64(DX)))   # hc = (SQC/hsq)^2

    nc = bacc.Bacc(None, target_bir_lowering=False, debug=True)
    d_misc = nc.dram_tensor("misc", [128, MISCW], DT, kind="ExternalInput")
    d_dmat = nc.dram_tensor("dmat", [128, 6 * 3 * 128], BF, kind="ExternalInput")
    d_out = nc.dram_tensor("out", [LROWS, RES * S27], DT, kind="ExternalOutput")

    # ext-block chunks and the local tiles each unlocks (tile t reads ext
    # blocks t..t+2, hsq block t+1, d block t)
    CHUNKS = [(range(0, 4), range(0, 2)),
              (range(4, 8), range(2, 6)),
              (range(8, 11), range(6, 9))]

    with tile.TileContext(nc) as tc:
        with (
            tc.tile_pool(name="const", bufs=1) as cpool,
            tc.tile_pool(name="work", bufs=3) as wpool,
            tc.tile_pool(name="wout", bufs=4) as wopool,
            tc.tile_pool(name="obuf", bufs=1) as opool,
            tc.tile_pool(name="psum", bufs=4, space="PSUM") as pspool,
        ):
            mi = cpool.tile([128, MISCW], DT)
            nc.sync.dma_start(mi[:], d_misc[:])
            dm = cpool.tile([128, 6 * 3 * 128], BF)
            nc.sync.dma_start(dm[:], d_dmat[:])

            HSQ = cpool.tile([128, HW_], DT)          # psi^2 (h = HSQ^2)
            HB = cpool.tile([128, HW_], BF)           # h in bf16
            HB3 = HB[:].rearrange("p (b z) -> p b z", z=RES)
            D = cpool.tile([128, NT * RES], DT)       # fwd z-diffs of h
            D3 = D[:].rearrange("p (t z) -> p t z", z=RES)

            # rotating output buffers (slot-major: free = s*96+z), zero
            # slots {5,7,11,15,19,21} pre-filled once, never rewritten
            otiles = []
            for i in range(NOB):
                O = opool.tile([128, RES * S27], DT, tag=f"ob{i}")
                OS = O[:].rearrange("p (s z) -> p s z", z=RES)
                eng = (nc.vector, nc.gpsimd, nc.vector, nc.gpsimd)[i]
                eng.memset(OS[:, 5:8:2, :], 0.0)
                eng.memset(OS[:, 11:20:4, :], 0.0)
                eng.memset(OS[:, 21, :], 0.0)
                otiles.append((O, OS))

            mh1 = mi[:, M_MH1:M_MH1 + 1]
            mh2 = mi[:, M_MH2:M_MH2 + 1]
            crow1 = mi[:, M_CROW1:M_CROW1 + RES]
            crow2 = mi[:, M_CROW2:M_CROW2 + RES]

            def emit_chunk(ci):
                blocks, tiles_d = CHUNKS[ci]
                b0, bn = blocks[0], len(blocks)
                csl = slice(RES * b0, RES * (b0 + bn))
                R1 = wpool.tile([128, bn * RES], DT, tag="r1")
                R2 = wpool.tile([128, bn * RES], DT, tag="r2")
                for e in blocks:
                    o = (e - b0) * RES
                    nc.scalar.activation(R1[:, o:o + RES], crow1, AF.Sqrt,
                                         bias=mi[:, M_AB1 + e:M_AB1 + e + 1])
                    nc.scalar.activation(R2[:, o:o + RES], crow2, AF.Sqrt,
                                         bias=mi[:, M_AB2 + e:M_AB2 + e + 1])
                Q1 = wpool.tile([128, bn * RES], DT, tag="q1")
                nc.vector.reciprocal_approx_fast(Q1[:], R1[:])
                Q2 = wpool.tile([128, bn * RES], DT, tag="q2")
                nc.vector.reciprocal_approx_fast(Q2[:], R2[:])
                # psi = mh1*q1 + mh2*q2 + 1 ; hsq = psi^2 ; h = hsq^2 (bf16)
                A = wpool.tile([128, bn * RES], DT, tag="aa")
                nc.vector.tensor_scalar(A[:], Q1[:], mh1, 1.0,
                                        AluOpType.mult, AluOpType.add)
                B = wpool.tile([128, bn * RES], DT, tag="bb")
                nc.vector.tensor_scalar(B[:], Q2[:], mh2, None, AluOpType.mult)
                PSI = wpool.tile([128, bn * RES], DT, tag="psi")
                nc.gpsimd.tensor_add(PSI[:], A[:], B[:])
                nc.vector.tensor_mul(HSQ[:, csl], PSI[:], PSI[:])
                nc.gpsimd.tensor_mul(HB[:, csl], HSQ[:, csl], HSQ[:, csl])
                # forward z-diffs for the local tiles this chunk covers
                ta, tb = tiles_d[0], tiles_d[-1]
                nc.gpsimd.tensor_sub(
                    D3[:, ta:tb + 1, 0:RES - 1],
                    HB3[:, ta + 1:tb + 2, 1:RES],
                    HB3[:, ta + 1:tb + 2, 0:RES - 1],
                )

            def emit_tile(t):
                g0e = 0 if t == 0 else (2 if t == NT - 1 else 1)
                g1e = 3 + (t % 3)
                p0 = pspool.tile([128, RES], DT, tag="p0")
                p1 = pspool.tile([128, RES], DT, tag="p1")
                for ge, pp in ((g0e, p0), (g1e, p1)):
                    for j in range(3):
                        lhs = dm[:, (ge * 3 + j) * 128:(ge * 3 + j + 1) * 128]
                        rsl = slice(RES * (t + j), RES * (t + j + 1))
                        nc.tensor.matmul(pp[:], lhs, HB[:, rsl],
                                         start=(j == 0), stop=(j == 2))

                hsl = slice(RES * (t + 1), RES * (t + 2))
                vinv = wopool.tile([128, RES], DT, tag="vinv")
                nc.vector.reciprocal_approx_fast(vinv[:], HSQ[:, hsl])
                hc = wopool.tile([128, RES], DT, tag="hc")
                nc.scalar.activation(hc[:], vinv[:], AF.Square, scale=SQC)

                # z central diffs from forward diffs (uniform hc scale)
                ST = wopool.tile([128, RES], DT, tag="st")
                nc.gpsimd.tensor_add(ST[:, 1:95], D3[:, t, 1:95], D3[:, t, 0:94])
                nc.gpsimd.tensor_scalar_mul(
                    ST[:, 0:RES:RES - 1], D[:, RES * t:RES * t + 95:94], 2.0
                )

                O, OS = otiles[t % NOB]
                # slot-major output: every write below is a dense 96-run.
                # W_c into slots 0..2 (PSUM sources must stay off GpSimd)
                nc.vector.tensor_mul(OS[:, 0, :], p0[:], hc[:])
                nc.vector.tensor_mul(OS[:, 1, :], p1[:], hc[:])
                nc.gpsimd.tensor_mul(OS[:, 2, :], ST[:], hc[:])
                # diagonal i=1,2 blocks (slots 12-14, 24-26) in one copy
                dap = O[:, 1152:1440]
                ddst = _dc.replace(dap, ap=[dap.ap[0], [1152, 2], [1, 288]])
                sap = O[:, 0:288]
                dsrc = _dc.replace(sap, ap=[sap.ap[0], [0, 2], [1, 288]])
                nc.vector.tensor_copy(ddst, dsrc)

                # off-diagonal slot pairs, sources = slots 0..2
                # (stride-0 sources are fast on Vector/Scalar, never GpSimd)
                def src2(c):
                    ap = OS[:, c, :]
                    return _dc.replace(ap, ap=[ap.ap[0], [0, 2]] + ap.ap[1:])
                nc.scalar.copy(OS[:, 10:21:10, :], src2(0))         # +W0
                nc.scalar.mul(OS[:, 4:9:4, :], src2(0), -1.0)       # -W0
                nc.vector.tensor_copy(OS[:, 3:24:20, :], src2(1))   # +W1
                nc.vector.tensor_scalar_mul(OS[:, 9:18:8, :], src2(1), -1.0)
                nc.scalar.copy(OS[:, 6:17:10, :], src2(2))          # +W2
                nc.scalar.mul(OS[:, 18:23:4, :], src2(2), -1.0)     # -W2

                nc.sync.dma_start(d_out[128 * t:128 * (t + 1), :], O[:])

            emit_chunk(0)
            emit_tile(0)
            emit_tile(1)
            emit_chunk(1)
            for t in range(2, 6):
                emit_tile(t)
            emit_chunk(2)
            for t in range(6, NT):
                emit_tile(t)

    nc.finalize()
    return nc


_CACHE = {}


def _get_setup():
    if "nc" not in _CACHE:
        x, DX = _grid_x()
        _CACHE["x"] = x
        _CACHE["dmat"] = [_build_dmat(c) for c in range(N_CORES)]
        _CACHE["xy"] = [_core_xy(c, x) for c in range(N_CORES)]
        _CACHE["nc"] = _build_program(DX)
    return _CACHE


def _in_maps(BH_positions, BH_masses_presoftplus):
    cache = _get_setup()
    x = cache["x"].astype(np.float64)
    pos = np.asarray(BH_positions, np.float32).astype(np.float64)
    pre = np.asarray(BH_masses_presoftplus, np.float32)
    masses = np.log1p(np.exp(pre)).astype(np.float32).astype(np.float64)
    mh = 0.5 * masses
    crow = [(x - pos[n, 2]) ** 2 for n in range(2)]

    maps = []
    for c in range(N_CORES):
        xcol, ycol = cache["xy"][c]
        misc = np.zeros((128, MISCW), np.float32)
        misc[:, M_CROW1:M_CROW1 + RES] = crow[0][None, :]
        misc[:, M_CROW2:M_CROW2 + RES] = crow[1][None, :]
        misc[:, M_AB1:M_AB1 + EXTNT] = (
            (xcol - pos[0, 0]) ** 2 + (ycol - pos[0, 1]) ** 2
        )
        misc[:, M_AB2:M_AB2 + EXTNT] = (
            (xcol - pos[1, 0]) ** 2 + (ycol - pos[1, 1]) ** 2
        )
        misc[:, M_MH1] = mh[0]
        misc[:, M_MH2] = mh[1]
        maps.append({"misc": misc, "dmat": cache["dmat"][c]})
    return cache["nc"], maps


def kernel(BH_positions, BH_masses_presoftplus):
    from concourse.bass_utils import run_bass_kernel_spmd

    nc, in_maps = _in_maps(BH_positions, BH_masses_presoftplus)
    res = run_bass_kernel_spmd(nc, in_maps, list(range(N_CORES)))
    # device rows are [a0, a1] x slot-major free (s*96 + a2); permute on host
    out = np.empty((RES, RES, RES, 3, 3, 3), np.float32)
    ov = out.reshape(N_CORES, PLANES, RES, RES, S27)
    for c in range(N_CORES):
        part = res.results[c]["out"].reshape(PLANES, RES, S27, RES)
        ov[c] = part.transpose(0, 1, 3, 2)
    return out
